# revision 3
# baseline (speedup 1.0000x reference)
"""Trainium2 Bass kernel for nn_BiLSTM_58351425683848.

Self-contained: accepts the FULL inputs of reference.setup_inputs(), returns
the FULL [256, 1024] output.  Seq-rows are sharded across 8 NeuronCores (32
rows each, activations packed as [128 H-partitions, 8 chunks x 32 seq]); the
only cross-core traffic is the BatchNorm statistics (two 8KB AllGathers per
step, pipelined against the GEMMs).  Only the live part of the reference is
computed: W4/b4, the LSTM cell updates and all but the last per-step output
are dead code, so the result is 0.5*(hf2+hb2) at t=255 of the
interaction/BatchNorm recurrence.

GEMM scheme (per core, per step, 3 GEMMs of [32,1024]x[1024,1024]):
- fp16 hi/lo split arithmetic (3 products) for fp32-equivalent accuracy at
  fp16 PE throughput.
- Activations are the PE-stationary operand ([K=128, M=32] tiles); weights
  stream as the moving operand (N=512 halves).  The four 32-wide stationary
  tiles sit at distinct PE column-groups (tile_position), and matmuls are
  issued wave-major across the groups so the four moving streams overlap
  (~4x PE throughput vs group-major issue).
- nh=0 half of all units is issued before nh=1 so the first PSUM half can
  be strip-copied (fp16 hi/lo) while the second half computes; the 4-strip
  partial sums are reduced+transposed back to the packed activation layout
  by small fp16 E-matmuls, with the bias added via a K=2 ones-row matmul.
- Sigmoid and BN statistics are emitted per half; BN apply is fused with
  the next step's (X + hf) hi/lo split, also per half, so the next GEMM's
  first waves start as early as possible after the stats AllGather lands.
- rsqrt(var+eps) runs entirely on the DVE (bit-trick seed + 2 Newton
  steps) so the ACT engine never swaps activation-function tables.
- GEMM matmuls issued wave-major across the 4 PE column-groups so the four
  32-wide stationary tiles stream concurrently (4x PE throughput on HW).
- nh=0 half of all units issued before nh=1, so the first PSUM half can be
  strip-copied while the second half computes.
- Strip reduction (4 col-groups -> packed layout) via fp16 hi/lo E-matmuls
  (16 small MMs) instead of fp32 E-matmuls.
- Bias folded into the sigmoid (ACT per-chunk bias) instead of a K=2 bias
  matmul wave.
Everything else (seq-row sharding over 8 cores, fp16 hi/lo split GEMM
arithmetic, BatchNorm stats AllGather pipelining) matches v1.
"""
import sys
sys.path.insert(0, '/opt/trn_rl_repo')
import numpy as np

S = 256
H = 1024
EPS = 1e-5
NK = 8
SL = 32


def pack_actT(a):
    sl = a.shape[0]
    out = np.empty((128, NK * sl), a.dtype)
    for k in range(NK):
        out[:, k * sl:(k + 1) * sl] = a[:, k * 128:(k + 1) * 128].T
    return np.ascontiguousarray(out)


def unpack_actT(p, sl=SL):
    a = np.empty((sl, H), p.dtype)
    for k in range(NK):
        a[:, k * 128:(k + 1) * 128] = p[:, k * sl:(k + 1) * sl].T
    return a


def pack_w_moving(w):
    out = np.empty((128, NK * H), w.dtype)
    for k in range(NK):
        out[:, k * H:(k + 1) * H] = w[:, k * 128:(k + 1) * 128].T
    return np.ascontiguousarray(out)


def pack_vec(v):
    return np.ascontiguousarray(v.reshape(NK, 128).T)


def split16(x):
    hi = x.astype(np.float16)
    lo = (x - hi.astype(np.float32)).astype(np.float16)
    return hi, lo


def build_kernel(nsteps, n_cores=8, debug_taps=()):
    import sys
    sys.path.insert(0, '/opt/trn_rl_repo')
    import concourse.bacc as bacc
    import concourse.tile as tile
    import concourse.mybir as mybir

    f32 = mybir.dt.float32
    f16 = mybir.dt.float16
    AFT = mybir.ActivationFunctionType
    ALU = mybir.AluOpType

    nc = bacc.Bacc("TRN2", target_bir_lowering=False, debug=False,
                   num_devices=n_cores)

    xt = nc.dram_tensor("xt", [128, NK * SL], f32, kind="ExternalInput")
    w_in = {}
    for nm in ("w1h", "w1l", "w2h", "w2l", "w3h", "w3l"):
        w_in[nm] = nc.dram_tensor(nm, [128, NK * H], f16, kind="ExternalInput")
    # bias rows: (b_hi; b_lo) [2, 1024] per gemm -> packed [2, 3*1024] fp16
    brow_in = nc.dram_tensor("brow", [2, 3 * H], f16, kind="ExternalInput")
    vecs_in = nc.dram_tensor("vecs", [128, NK * 4], f32, kind="ExternalInput")
    outp = nc.dram_tensor("out", [128, NK * SL], f32, kind="ExternalOutput")
    taps = {}
    for nm in debug_taps:
        taps[nm] = nc.dram_tensor(f"tap_{nm}", [128, NK * SL], f32,
                                  kind="ExternalOutput")

    E_np = np.tile(np.eye(SL, dtype=np.float16), (4, 1))
    e_dram = nc.inline_tensor(E_np, name="emat")
    ones2_np = np.ones((2, SL), dtype=np.float16)
    ones2_dram = nc.inline_tensor(ones2_np, name="ones2")

    with tile.TileContext(nc) as tc:
        with tc.tile_pool(name="wpool", bufs=1) as wpool, \
             tc.tile_pool(name="spool", bufs=3) as spool, \
             tc.tile_pool(name="dpool", bufs=4, space="DRAM") as dpool, \
             tc.tile_pool(name="ppool", bufs=2, space="PSUM") as ppool, \
             tc.tile_pool(name="pxpool", bufs=2, space="PSUM") as pxpool, \
             tc.tile_pool(name="warmp", bufs=1, space="PSUM") as warmp:

            w_sb = {}
            for nm in w_in:
                w_sb[nm] = wpool.tile([128, NK * H], f16, tag=nm, name=nm)
                for k in range(NK):
                    nc.sync.dma_start(w_sb[nm][:, k * H:(k + 1) * H],
                                      w_in[nm][:, k * H:(k + 1) * H])
            vecs = wpool.tile([128, NK * 4], f32, tag="vecs")
            nc.sync.dma_start(vecs[:], vecs_in[:])
            gfp = vecs[:, 0 * NK:1 * NK]
            bfp = vecs[:, 1 * NK:2 * NK]
            gbp = vecs[:, 2 * NK:3 * NK]
            bbp = vecs[:, 3 * NK:4 * NK]
            brow = wpool.tile([2, 3 * H], f16, tag="brow")
            nc.sync.dma_start(brow[:], brow_in[:])
            bias_g = {"1": brow[:, 0:H], "2": brow[:, H:2 * H],
                      "3": brow[:, 2 * H:3 * H]}
            e_sb = wpool.tile([128, SL], f16, tag="emat")
            nc.sync.dma_start(e_sb[:], e_dram[:])
            ones2 = wpool.tile([2, SL], f16, tag="ones2")
            nc.sync.dma_start(ones2[:], ones2_dram[:])
            xt_sb = wpool.tile([128, NK * SL], f32, tag="xt")
            nc.sync.dma_start(xt_sb[:], xt[:])
            epsc = wpool.tile([128, 1], f32, tag="epsc")
            nc.vector.memset(epsc[:], EPS)

            hfT = wpool.tile([128, NK * SL], f32, tag="hfT")
            hbT = wpool.tile([128, NK * SL], f32, tag="hbT")
            nc.vector.memset(hfT[:], 0.0)
            nc.vector.memset(hbT[:], 0.0)

            PW = NK * SL

            def add_split(x, y, tagbase):
                """(ah, al) = fp16 hi/lo of (x + y), emitted per half so the
                consumer GEMM's first waves can start after half 0."""
                ah = spool.tile([128, PW], f16, tag=tagbase + "h",
                                name=tagbase + "h")
                al = spool.tile([128, PW], f16, tag=tagbase + "l",
                                name=tagbase + "l")
                tr = spool.tile([128, PW], f32, tag="addres", name="addres")
                hw = PW // 2
                for m in range(2):
                    sl_ = slice(m * hw, (m + 1) * hw)
                    nc.vector.tensor_add(ah[:, sl_], x[:, sl_], y[:, sl_])
                    nc.vector.tensor_sub(tr[:, sl_], x[:, sl_], ah[:, sl_])
                    nc.vector.tensor_add(al[:, sl_], tr[:, sl_], y[:, sl_])
                return ah, al

            def gemm(ah, al, wh, wl, gi, xout):
                """xout <- sigmoid((ah+al)@(Wh+Wl) + b) in packed layout.

                MMs wave-major across 4 col-groups; nh=0 half first; strip
                reduce via fp16 hi/lo E-matmuls; sigmoid with ACT bias.
                """
                P = ppool.tile([128, H], f32, tag="P", name="P" + gi)
                units = [(k, p) for k in range(NK) for p in range(3)]
                per_group = [[] for _ in range(4)]
                for ui, u in enumerate(units):
                    per_group[ui % 4].append(u)
                depth = len(per_group[0])  # 6
                for nh in range(2):
                    for idx in range(depth):
                        for g in range(4):
                            k, p = per_group[g][idx]
                            lhs = (ah if p in (0, 1) else al)
                            w = (w_sb[wh] if p in (0, 2) else w_sb[wl])
                            nc.tensor.matmul(
                                P[32 * g:32 * (g + 1),
                                  512 * nh:512 * (nh + 1)],
                                lhs[:, k * SL:(k + 1) * SL],
                                w[:, k * H + 512 * nh:k * H + 512 * (nh + 1)],
                                start=(idx == 0), stop=(idx == depth - 1),
                                tile_position=(0, 32 * g),
                            )
                    # strip-copy this half while the other half's MMs run
                    if nh == 0:
                        Shi = spool.tile([128, H], f16, tag="Shi",
                                         name="Shi" + gi)
                        Slo = spool.tile([128, H], f16, tag="Slo",
                                         name="Slo" + gi)
                    nc.scalar.activation(Shi[:, 512 * nh:512 * (nh + 1)],
                                         P[:, 512 * nh:512 * (nh + 1)],
                                         AFT.Copy)
                    nc.vector.tensor_sub(Slo[:, 512 * nh:512 * (nh + 1)],
                                         P[:, 512 * nh:512 * (nh + 1)],
                                         Shi[:, 512 * nh:512 * (nh + 1)])
                px = pxpool.tile([128, PW], f32, tag="px", name="px" + gi)
                bias = bias_g[gi]
                for j in range(NK):
                    nc.tensor.matmul(px[:, j * SL:(j + 1) * SL],
                                     bias[:, j * 128:(j + 1) * 128],
                                     ones2[:], start=True, stop=False)
                    nc.tensor.matmul(px[:, j * SL:(j + 1) * SL],
                                     Shi[:, j * 128:(j + 1) * 128],
                                     e_sb[:], start=False, stop=False)
                    nc.tensor.matmul(px[:, j * SL:(j + 1) * SL],
                                     Slo[:, j * 128:(j + 1) * 128],
                                     e_sb[:], start=False, stop=True)
                    if j == NK // 2 - 1:
                        nc.scalar.activation(xout[:, 0:PW // 2],
                                             px[:, 0:PW // 2], AFT.Sigmoid)
                nc.scalar.activation(xout[:, PW // 2:PW],
                                     px[:, PW // 2:PW], AFT.Sigmoid)
                return px

            def stats_of(hx2, tag):
                """Per-chunk sums and sums-of-squares, emitted per half."""
                st = spool.tile([128, 16], f32, tag="st" + tag, name="st" + tag)
                sq_ = spool.tile([128, PW], f32, tag="sqscr", name="sq" + tag)
                hw = PW // 2
                hj = NK // 2
                for m in range(2):
                    sl_ = slice(m * hw, (m + 1) * hw)
                    nc.vector.tensor_reduce(
                        st[:, m * hj:(m + 1) * hj],
                        hx2[:, sl_].rearrange("p (j s) -> p j s", j=hj),
                        axis=mybir.AxisListType.X, op=ALU.add)
                    nc.scalar.activation(sq_[:, sl_], hx2[:, sl_], AFT.Square)
                    nc.vector.tensor_reduce(
                        st[:, 8 + m * hj:8 + (m + 1) * hj],
                        sq_[:, sl_].rearrange("p (j s) -> p j s", j=hj),
                        axis=mybir.AxisListType.X, op=ALU.add)
                return st

            def launch_ag(st, tag):
                inb = dpool.tile([128, 16], f32, tag="agi" + tag,
                                 name="agi" + tag)
                outb = dpool.tile([128 * n_cores, 16], f32, tag="ago" + tag,
                                  name="ago" + tag)
                nc.sync.dma_start(inb[:], st[:])
                nc.gpsimd.collective_compute(
                    "AllGather", ALU.bypass,
                    replica_groups=[list(range(n_cores))],
                    ins=[inb.opt()], outs=[outb.opt()],
                )
                return outb

            def bn_apply(outb, gamma, beta, hx2, hxT, tag, fuse=None):
                gath = spool.tile([128, n_cores * 16], f32, tag="gath" + tag,
                                  name="gath" + tag)
                nc.sync.dma_start(
                    gath[:].rearrange("p (r c) -> p r c", r=n_cores),
                    outb[:].rearrange("(r p) c -> p r c", p=128))
                tot = spool.tile([128, 16], f32, tag="tot" + tag,
                                 name="tot" + tag)
                nc.vector.tensor_reduce(
                    tot[:], gath[:].rearrange("p (r c) -> p c r", r=n_cores),
                    axis=mybir.AxisListType.X, op=ALU.add)
                prm = spool.tile([128, 40], f32, tag="prm" + tag,
                                 name="prm" + tag)
                mean = prm[:, 0:8]
                var = prm[:, 8:16]
                a_ = prm[:, 16:24]
                c_ = prm[:, 24:32]
                msq = prm[:, 32:40]
                nc.vector.tensor_scalar_mul(mean, tot[:, 0:8], 1.0 / S)
                nc.vector.tensor_mul(msq, mean, mean)
                nc.vector.tensor_scalar(var, tot[:, 8:16], 1.0 / S, None,
                                        ALU.mult)
                nc.vector.tensor_sub(var, var, msq)
                # rsqrt(var+eps) on DVE (bit-trick seed + 2 Newton steps) so
                # the ACT engine never loads the sqrt function table
                u_ = spool.tile([128, 8], f32, tag="u" + tag, name="u" + tag)
                nc.vector.tensor_scalar_add(u_[:], var, EPS)
                ri = spool.tile([128, 8], mybir.dt.int32, tag="ri" + tag,
                                name="ri" + tag)
                nc.vector.tensor_scalar(ri[:], u_[:].bitcast(mybir.dt.int32),
                                        1, None, ALU.logical_shift_right)
                nc.vector.tensor_scalar(ri[:], ri[:], -1, 0x5F3759DF,
                                        ALU.mult, ALU.add)
                r0 = ri[:].bitcast(f32)
                nr = spool.tile([128, 8], f32, tag="nr" + tag,
                                name="nr" + tag)
                for _ in range(2):
                    nc.vector.tensor_mul(nr[:], r0, r0)
                    nc.vector.tensor_mul(nr[:], nr[:], u_[:])
                    nc.vector.tensor_scalar(nr[:], nr[:], -0.5, 1.5,
                                            ALU.mult, ALU.add)
                    nc.vector.tensor_mul(r0, r0, nr[:])
                nc.vector.tensor_mul(a_, gamma, r0)
                nc.vector.tensor_mul(c_, a_, mean)
                nc.vector.tensor_sub(c_, beta, c_)
                hw = PW // 2
                for m in range(2):
                    for j in range(m * NK // 2, (m + 1) * NK // 2):
                        nc.vector.tensor_scalar(
                            hxT[:, j * SL:(j + 1) * SL],
                            hx2[:, j * SL:(j + 1) * SL],
                            a_[:, j:j + 1], c_[:, j:j + 1],
                            ALU.mult, ALU.add)
                    if fuse is not None:
                        # split of (x_other + hxT) for this half, so the next
                        # GEMM's first waves start before the second half
                        x_o, ah, al, tr = fuse
                        sl_ = slice(m * hw, (m + 1) * hw)
                        nc.vector.tensor_add(ah[:, sl_], x_o[:, sl_],
                                             hxT[:, sl_])
                        nc.vector.tensor_sub(tr[:, sl_], x_o[:, sl_],
                                             ah[:, sl_])
                        nc.vector.tensor_add(al[:, sl_], tr[:, sl_],
                                             hxT[:, sl_])

            pend_b = None
            s1h = s1l = None
            for t in range(nsteps):
                last = (t == nsteps - 1)
                if s1h is None:
                    a1h, a1l = add_split(xt_sb, hfT, "a1s")
                else:
                    a1h, a1l = s1h, s1l  # produced fused with bn_f last step
                x1 = spool.tile([128, PW], f32, tag="x1")
                gemm(a1h, a1l, "w1h", "w1l", "1", x1)

                if pend_b is not None:
                    outb_b, hb2_prev = pend_b
                    bn_apply(outb_b, gbp, bbp, hb2_prev, hbT, "b")
                    pend_b = None

                a3h, a3l = add_split(x1, hfT, "a3s")
                hf2 = spool.tile([128, PW], f32, tag="hf2")
                gemm(a3h, a3l, "w3h", "w3l", "3", hf2)
                if not last:
                    st_f = stats_of(hf2, "f")
                    outb_f = launch_ag(st_f, "f")

                a2h, a2l = add_split(hbT, x1, "a2s")
                hb2 = spool.tile([128, PW], f32, tag="hb2")
                gemm(a2h, a2l, "w2h", "w2l", "2", hb2)

                if last:
                    o = spool.tile([128, PW], f32, tag="o")
                    nc.vector.tensor_add(o[:], hf2[:], hb2[:])
                    nc.vector.tensor_scalar_mul(o[:], o[:], 0.5)
                    nc.sync.dma_start(outp[:], o[:])
                    for nm, ap in (("x1", x1), ("hf2", hf2), ("hb2", hb2)):
                        if nm in taps:
                            nc.sync.dma_start(taps[nm][:], ap[:])
                    continue

                st_b = stats_of(hb2, "b")
                outb_b = launch_ag(st_b, "b")
                pend_b = (outb_b, hb2)

                # keep-warm MMs so the PE clock stays hot across the AG_f wait
                wp = warmp.tile([128, 512], f32, tag="wp", name="wp")
                for d in range(12):
                    nc.tensor.matmul(wp[0:32, :],
                                     a2h[:, (d % NK) * SL:((d % NK) + 1) * SL],
                                     w_sb["w1h"][:, 0:512],
                                     start=True, stop=True,
                                     skip_group_check=True)
                wscr = spool.tile([128, 8], f32, tag="wscr", name="wscr")
                nc.vector.tensor_copy(wscr[:32, :], wp[0:32, 0:8])

                s1h = spool.tile([128, PW], f16, tag="a1sh", name="a1sh")
                s1l = spool.tile([128, PW], f16, tag="a1sl", name="a1sl")
                s1tr = spool.tile([128, PW], f32, tag="addres", name="s1tr")
                bn_apply(outb_f, gfp, bfp, hf2, hfT, "f",
                         fuse=(xt_sb, s1h, s1l, s1tr))

    nc.compile()
    return nc


def numpy_sim(inp, nsteps):
    sig = lambda x: 1.0 / (1.0 + np.exp(-x))

    def bn(x, g, b):
        m = x.mean(0)
        xc = x - m
        v = (xc * xc).mean(0)
        return xc / np.sqrt(v + EPS) * g + b

    X = inp["inputs"]
    hf = np.zeros((S, H), np.float32)
    hb = np.zeros((S, H), np.float32)
    for t in range(nsteps):
        x1 = sig((X + hf) @ inp["W1"].T + inp["b1"])
        hb2 = sig((hb + x1) @ inp["W2"].T + inp["b2"])
        hf2 = sig((x1 + hf) @ inp["W3"].T + inp["b3"])
        out = (hf2 + hb2) * 0.5
        hf = bn(hf2, inp["gamma_f"], inp["beta_f"])
        hb = bn(hb2, inp["gamma_b"], inp["beta_b"])
    return out, x1, hf2, hb2


def make_in_maps(inp, n_cores=8):
    m = {}
    for i, wn in enumerate(("W1", "W2", "W3")):
        wh, wl = split16(np.asarray(inp[wn], np.float32))
        m[f"w{i+1}h"] = pack_w_moving(wh)
        m[f"w{i+1}l"] = pack_w_moving(wl)
    brow = np.zeros((2, 3 * H), np.float16)
    for i, bn_ in enumerate(("b1", "b2", "b3")):
        bh, bl = split16(np.asarray(inp[bn_], np.float32))
        brow[0, i * H:(i + 1) * H] = bh
        brow[1, i * H:(i + 1) * H] = bl
    m["brow"] = brow
    vecs = np.zeros((128, NK * 4), np.float32)
    for i, nm in enumerate(("gamma_f", "beta_f", "gamma_b", "beta_b")):
        vecs[:, i * NK:(i + 1) * NK] = pack_vec(np.asarray(inp[nm], np.float32))
    m["vecs"] = vecs
    X = np.asarray(inp["inputs"], np.float32)
    maps = []
    for c in range(n_cores):
        mm = dict(m)
        mm["xt"] = pack_actT(X[c * SL:(c + 1) * SL, :])
        maps.append(mm)
    return maps


def assemble_out(results, n_cores=8):
    out = np.empty((S, H), np.float32)
    for c in range(n_cores):
        out[c * SL:(c + 1) * SL, :] = unpack_actT(results[c]["out"])
    return out


_NC_CACHE = {}


def kernel(**inputs):
    import numpy as np
    nsteps = S
    key = nsteps
    if key not in _NC_CACHE:
        _NC_CACHE[key] = build_kernel(nsteps)
    nc = _NC_CACHE[key]
    inp = {k: np.asarray(v) for k, v in inputs.items()}
    maps = make_in_maps(inp)
    from concourse.bass_utils import run_bass_kernel_spmd
    res = run_bass_kernel_spmd(nc, maps, core_ids=list(range(8)))
    return assemble_out(res.results).astype(np.float32)
